# revision 2
# baseline (speedup 1.0000x reference)
"""Trainium2 Bass kernel v2 for the BKT (multi-HMM knowledge tracing) model.

Data-parallel over students (1024/8 = 128 per core, one SBUF partition each);
the T=500 recursion runs locally per core.

Reformulated recursion (per student, per step t):
    a2_j   = <c_t, alpha_j>                       j in {0,1} (state axis)
    d      = a2_1 - a2_0
    a3_i   = a2_0 + M40_i + ln(1 + e^{dM_i + d})  (lse over j, shifted by j=0)
    alpha' = (1-c) * alpha + c * a3               (per state block)
    lpy_o  = ln(EL0_o + e^{L1_o + d}) - ln(1+e^d) (outputs; depend only on d!)

with host-precomputed gather tables:
    M40_i = G4[i,0]+OLL_0, dM_i = G4[i,1]-G4[i,0]+OLL_1-OLL_0,
    G4 = A @ log_t, OLL_j = log_obs[p, j, corr], EL0_o = e^{L[p,0,o]},
    L1_o = L[p,1,o].

State kept as gamma = alpha - a3_prev (fp16), giving the fused updates
    a2_j    = a3p_j + <c, gamma_j>                (one ttr per j, seed=a3p_j)
    gamma'_ = ((gamma + (a3p - a3new)) * (1-c))   (one stt per state block)

Outputs are NOT computed in the loop: d and ln(1+e^d) stream to SBUF and a
short bulk epilogue (exp/ln over the archived L-columns) produces log_py.

Per step: 7 DVE + 2 Act instructions, 2 cross-engine handoffs.
"""

import os
from contextlib import ExitStack

import numpy as np

N_PROBLEMS = 10000
N_KCS = 100
BATCH = 1024
T_FULL = 500
N_CORES = 8
BL = BATCH // N_CORES  # 128 students per core

_CHUNK = 8  # time steps per gather slab (8*128 = 1024 = SWDGE ring capacity)
_EP = 125  # epilogue chunk width


def _log_softmax(x, axis):
    x = x.astype(np.float32)
    m = x.max(axis=axis, keepdims=True)
    e = np.exp(x - m)
    return (x - m) - np.log(e.sum(axis=axis, keepdims=True))


def _wrap_idx(flat):
    """dma_gather index layout: flat index i lives at partition i%16, col i//16,
    replicated across the 8 gpsimd cores (16-partition groups)."""
    assert flat.size % 16 == 0
    w = flat.astype(np.int16).reshape(-1, 16).T  # [16, N/16]
    return np.tile(w, (8, 1))  # [128, N/16]


def _host_tables(A, trans_logits, obs_logits_problem, init_logits):
    P = A.shape[0]
    K = trans_logits.shape[0]
    log_t = _log_softmax(trans_logits, axis=1)  # [K,2,2] normalized over dim 1
    G = A.astype(np.float32) @ log_t.reshape(K, 4)  # [P,4] pairs (i,j)
    L = _log_softmax(obs_logits_problem, axis=2)  # [P,2,2] normalized over o

    # taba (kc-indexed), f32 1KB rows:
    # cols [0:100] = A, [100:104] = G4 (i,j) pairs, [128:228] = 1 - A
    taba = np.zeros((P, 256), np.float32)
    taba[:, 0:100] = A.astype(np.float32)
    taba[:, 100:104] = G
    taba[:, 128:228] = (1.0 - A).astype(np.float32)

    # tabp row (2*problem + corr), f32 256B rows:
    #  [0]=OLL_0 [1]=OLL_1 | [2:4]=EL0_o | [4:6]=L1_o
    tabp = np.zeros((2 * P, 64), np.float32)
    for corr in (0, 1):
        tabp[corr::2, 0] = L[:, 0, corr]
        tabp[corr::2, 1] = L[:, 1, corr]
    tabp[:, 2:4] = np.exp(np.repeat(L[:, 0, :], 2, axis=0))  # EL0_o
    tabp[:, 4:6] = np.repeat(L[:, 1, :], 2, axis=0)          # L1_o

    la0 = _log_softmax(init_logits, axis=1)  # [K,2]
    g0 = np.empty((BL, 2 * K + 2), np.float32)
    g0[:, 0:K] = la0[:, 0]  # state block j=0
    g0[:, K:2 * K] = la0[:, 1]  # j=1
    g0[:, 2 * K:] = 0.0  # a3p seed zeros
    return taba, tabp, g0, g0[:, 2 * K:].copy()


def _setup_act_tables():
    """Make bacc+walrus agree that Exp and Ln share one ACT table set, so the
    kernel does a single ACT_TABLE_LOAD instead of one per activation."""
    import glob
    import json
    import tempfile

    if os.environ.get("_BKT_ACT_TABLES"):
        return
    from neuronxcc.driver.Job import Job  # pyright: ignore[reportMissingImports]
    from neuronxcc.driver.jobs.support.FindActInfo import (  # pyright: ignore[reportMissingImports]
        findActInfoFile,
    )

    src = findActInfoFile(Job.getPackageDir(), "gen3")
    d = json.load(open(src))
    d["act_func_sets"] = sorted(
        d["act_func_sets"],
        key=lambda s: s["name"] != "natural_log_exp_and_others")
    tmp = tempfile.mkdtemp(prefix="bkt_act_")
    with open(tmp + "/act_info.json", "w") as f:
        json.dump(d, f)
    for p in glob.glob(os.path.dirname(src) + "/*"):
        b = os.path.basename(p)
        if b != "act_info.json":
            os.symlink(p, tmp + "/" + b)
    os.environ["BASS_ACT_ROOT_JSON_PATH"] = tmp + "/act_info.json"
    os.environ["_BKT_ACT_TABLES"] = "1"

    import concourse.bacc as bacc_mod
    import concourse.mybir as mybir

    def tables(arch):
        out = {
            e["name"]: {mybir.ActivationFunctionType.from_pwp(v)
                        for v in e["act"].keys()}
            for e in d["act_func_sets"]
        }
        # act_info.json lists softplus as the generic 'act2' slot; teach the
        # bass-side chooser that this set implements Softplus.
        out["softplus_and_others"].add(mybir.ActivationFunctionType.Softplus)
        return out

    bacc_mod.get_activation_tables = tables


def _emit_program(T, Tc):
    import concourse.mybir as mybir
    import concourse.tile as tile
    from concourse import bacc

    _setup_act_tables()

    f32 = mybir.dt.float32
    f16 = mybir.dt.float16
    i16 = mybir.dt.int16
    Alu = mybir.AluOpType
    Act = mybir.ActivationFunctionType
    K = N_KCS

    nc = bacc.Bacc("TRN2", target_bir_lowering=False, debug=False,
                   dynamic_dma_scratch_size=65536, num_swdge_queues=2,
                   monotonic_sem_count=int(os.environ.get("_NSEM", "1")))

    taba = nc.dram_tensor("taba", [N_PROBLEMS, 256], f32, kind="ExternalInput")
    tabp = nc.dram_tensor("tabp", [2 * N_PROBLEMS, 64], f32, kind="ExternalInput")
    kcw = nc.dram_tensor("kcw", [128, T * 8], i16, kind="ExternalInput")
    ppw = nc.dram_tensor("ppw", [128, T * 8], i16, kind="ExternalInput")
    gam0 = nc.dram_tensor("gam0", [BL, 2 * K + 2], f32, kind="ExternalInput")
    a3p0 = nc.dram_tensor("a3p0", [BL, 2], f32, kind="ExternalInput")
    out = nc.dram_tensor("out", [BL, T * 2], f32, kind="ExternalOutput")

    assert Tc * 128 <= 1024  # SWDGE descriptor ring capacity per gather
    chunks = []
    t0 = 0
    while t0 < T:
        chunks.append((t0, min(Tc, T - t0)))
        t0 += Tc
    n_chunks = len(chunks)

    from concourse import library_config

    with ExitStack() as ctx:
        tc = ctx.enter_context(tile.TileContext(nc))
        nc.gpsimd.load_library(library_config.mlp)
        idx_pool = ctx.enter_context(tc.tile_pool(name="idx", bufs=1))
        slab_pool = ctx.enter_context(tc.tile_pool(name="slabs", bufs=int(os.environ.get("_BA", "4"))))
        slabp_pool = ctx.enter_context(tc.tile_pool(name="slabsp", bufs=int(os.environ.get("_BP", "8"))))
        m4_pool = ctx.enter_context(tc.tile_pool(name="m4p", bufs=3))
        state_pool = ctx.enter_context(tc.tile_pool(name="state", bufs=2))
        small_pool = ctx.enter_context(tc.tile_pool(name="small", bufs=2))
        big_pool = ctx.enter_context(tc.tile_pool(name="big", bufs=1))
        ep_pool = ctx.enter_context(tc.tile_pool(name="ep", bufs=2))

        kcw_t = idx_pool.tile([128, T * 8], i16, tag="kcw")
        nc.sync.dma_start(kcw_t[:], kcw.ap())
        ppw_t = idx_pool.tile([128, T * 8], i16, tag="ppw")
        nc.sync.dma_start(ppw_t[:], ppw.ap())

        # Per-chain state/scratch (chains: partitions 0:64 and 64:128).
        # Separate tiles per chain so the dep tracker never sees a
        # same-tile write-write conflict between the two chains.
        gamma, a3p, dbuf, l2buf, a2p, scr, e2 = {}, {}, {}, {}, {}, {}, {}
        for ci in range(2):
            gamma[ci] = [state_pool.tile([128, 2 * K], f32,
                                         tag=f"gam{ci}_{i}", name=f"gam{ci}_{i}")
                         for i in range(2)]
            a3p[ci] = [state_pool.tile([128, 2], f32, tag=f"a3p{ci}_{i}",
                                       name=f"a3p{ci}_{i}") for i in range(2)]
            dbuf[ci] = big_pool.tile([128, T], f32, tag=f"dbuf{ci}",
                                     name=f"dbuf{ci}")
            l2buf[ci] = big_pool.tile([128, 2 * T], f32, tag=f"l2buf{ci}",
                                      name=f"l2buf{ci}")
            a2p[ci] = big_pool.tile([128, 2], f32, tag=f"a2p{ci}",
                                    name=f"a2p{ci}")
            scr[ci] = big_pool.tile([128, K], f32, tag=f"scr{ci}",
                                    name=f"scr{ci}")
            e2[ci] = big_pool.tile([128, 2], f32, tag=f"e2{ci}",
                                   name=f"e2{ci}")
        NCH = int(os.environ.get("_NCHAINS", "1"))
        P_OF = {0: slice(0, 64), 1: slice(64, 128)} if NCH == 2 else {0: slice(0, 128)}
        for ci in P_OF:
            P = P_OF[ci]
            nc.sync.dma_start(gamma[ci][0][P], gam0.ap()[P, 0:2 * K])
            nc.sync.dma_start(a3p[ci][0][P], a3p0.ap()[P, :])
        Lbuf = big_pool.tile([128, T, 4], f32, tag="Lbuf")
        outbuf = big_pool.tile([128, T * 2], f32, tag="outbuf")

        slabsA = [None] * n_chunks
        slabsP = [None] * n_chunks
        m4s = [None] * n_chunks
        dms = [None] * n_chunks
        ni_regs = {}
        for tcn in sorted({c[1] for c in chunks}):
            r = nc.gpsimd.alloc_register(f"ni{tcn}")
            nc.gpsimd.reg_mov(r, tcn * 128)
            ni_regs[tcn] = r

        def issue_gather(n):
            t0, tcn = chunks[n]
            ni = ni_regs[tcn]
            sa = slab_pool.tile([128, Tc, 256], f32, tag="slabA")
            nc.gpsimd.dma_gather(
                sa[:, 0:tcn, :], taba.ap(), kcw_t[:, t0 * 8:(t0 + tcn) * 8],
                num_idxs=tcn * 128, num_idxs_reg=ni, elem_size=256,
                queue_num=0,
            )
            sp = slabp_pool.tile([128, Tc, 64], f32, tag="slabP")
            nc.gpsimd.dma_gather(
                sp[:, 0:tcn, :], tabp.ap(), ppw_t[:, t0 * 8:(t0 + tcn) * 8],
                num_idxs=tcn * 128, num_idxs_reg=ni, elem_size=64,
                queue_num=1,
            )
            slabsA[n], slabsP[n] = sa, sp

        def issue_m4(n):
            t0, tcn = chunks[n]
            sa, sp = slabsA[n], slabsP[n]
            m4 = m4_pool.tile([128, Tc, 4], f32, tag="m4", name="m4")
            # M4[i,j] = G4[i,j] + OLL_j  (broadcast over i)
            nc.vector.tensor_tensor(
                out=m4[:, 0:tcn, :].rearrange("p t (i j) -> p t i j", j=2),
                in0=sa[:, 0:tcn, 100:104].rearrange("p t (i j) -> p t i j", j=2),
                in1=sp[:, 0:tcn, 0:2].unsqueeze(2).broadcast_to(
                    [128, tcn, 2, 2]),
                op=Alu.add,
            )
            dm = m4_pool.tile([128, Tc, 2], f32, tag="dm", name="dm")
            m4v = m4[:, 0:tcn, :].rearrange("p t (i j) -> p t i j", j=2)
            # dM_i = M4[i,1] - M4[i,0]
            nc.vector.tensor_tensor(
                out=dm[:, 0:tcn, :], in0=m4v[:, :, :, 1], in1=m4v[:, :, :, 0],
                op=Alu.subtract,
            )
            m4s[n], dms[n] = m4, dm

        issue_gather(0)
        issue_gather(1)
        issue_gather(2)
        issue_m4(0)

        for n in range(n_chunks):
            t0, tcn = chunks[n]
            if n + 3 < n_chunks:
                issue_gather(n + 3)
            sa, sp = slabsA[n], slabsP[n]
            m4, dm = m4s[n], dms[n]
            m4v = m4[:, 0:Tc, :].rearrange("p t (i j) -> p t i j", j=2)
            # Two independent 64-partition chains (students 0-63 / 64-127)
            # interleaved so each chain's compute hides the other's
            # dependency stalls. Chains have private tiles; slabs are shared
            # read-only.
            def front(t, j, ci):
                """a2 pair + d for chain ci."""
                P = P_OF[ci]
                cur = a3p[ci][t % 2][P]
                gcur = gamma[ci][t % 2][P]
                c_ap = sa[P, j, 0:K]
                for s in range(2):
                    nc.vector.scalar_tensor_tensor(
                        out=scr[ci][P], in0=gcur[:, s * K:(s + 1) * K],
                        scalar=cur[:, s:s + 1], in1=c_ap,
                        op0=Alu.add, op1=Alu.mult,
                        accum_out=a2p[ci][P, s:s + 1],
                    )
                # d = a2_1 - a2_0
                nc.vector.tensor_tensor(
                    out=dbuf[ci][P, t:t + 1], in0=a2p[ci][P, 1:2],
                    in1=a2p[ci][P, 0:1], op=Alu.subtract,
                )

            def acts(t, j, ci):
                P = P_OF[ci]
                # e2 = exp([dM_0, dM_1] + d)
                nc.scalar.activation(
                    e2[ci][P], dm[P, j, 0:2], Act.Exp,
                    bias=dbuf[ci][P, t:t + 1],
                )
                # l2 = ln(e2 + 1) -> ln1p pair
                nc.scalar.activation(l2buf[ci][P, 2 * t:2 * t + 2],
                                     e2[ci][P], Act.Ln, bias=1.0)

            def back(t, j, ci, b2):
                P = P_OF[ci]
                cur = a3p[ci][t % 2][P]
                nxt = a3p[ci][(t + 1) % 2][P]
                gcur = gamma[ci][t % 2][P]
                gnxt = gamma[ci][(t + 1) % 2][P]
                cm_ap = sa[P, j, 128:128 + K]
                lg = l2buf[ci][P, 2 * t:2 * t + 2]
                # a3new_i = (ln1p_i + a2_0) + M40_i
                nc.vector.scalar_tensor_tensor(
                    out=nxt[:], in0=lg[:, 0:2], scalar=a2p[ci][P, 0:1],
                    in1=m4v[P, j, :, 0], op0=Alu.add, op1=Alu.add,
                )
                # b2 = a3p - a3new
                nc.vector.tensor_tensor(
                    out=b2[P], in0=cur[:], in1=nxt[:], op=Alu.subtract,
                )
                # gamma'_s = (gamma_s + b2_s) * (1 - c)
                for s in range(2):
                    nc.vector.scalar_tensor_tensor(
                        out=gnxt[:, s * K:(s + 1) * K],
                        in0=gcur[:, s * K:(s + 1) * K],
                        scalar=b2[P, s:s + 1], in1=cm_ap,
                        op0=Alu.add, op1=Alu.mult,
                    )

            for j in range(tcn):
                t = t0 + j
                b2a = small_pool.tile([128, 2], f32, tag="b2a", name="b2a")
                if NCH == 2:
                    b2b = small_pool.tile([128, 2], f32, tag="b2b", name="b2b")
                    front(t, j, 0)
                    acts(t, j, 0)
                    front(t, j, 1)
                    acts(t, j, 1)
                    back(t, j, 0, b2a)
                    back(t, j, 1, b2b)
                else:
                    front(t, j, 0)
                    acts(t, j, 0)
                    back(t, j, 0, b2a)
            if n + 1 < n_chunks:
                issue_m4(n + 1)
            # archive the epilogue columns [EL0_0, EL0_1, L1_0, L1_1] via an
            # SBUF->SBUF DMA on the idle SP queue (keeps all compute engines
            # out of the slab-lifetime dependency chain)
            nc.sync.dma_start(Lbuf[:, t0:t0 + tcn, :], sp[:, 0:tcn, 2:6])

        # ---- epilogue: log_py from d stream + archived L columns ----
        for ci in P_OF:
            P = P_OF[ci]
            ep0 = 0
            while ep0 < T:
                W = min(_EP, T - ep0)
                dv = dbuf[ci][P, ep0:ep0 + W]
                # zq = ln(1 + e^d)
                zqe = ep_pool.tile([128, _EP], f32, tag="zqe", name="zqe")
                nc.scalar.activation(zqe[P, 0:W], dv, Act.Exp)
                zqt = ep_pool.tile([128, _EP], f32, tag="zqt", name="zqt")
                nc.scalar.activation(zqt[P, 0:W], zqe[P, 0:W], Act.Ln,
                                     bias=1.0)
                zq = zqt[P, 0:W]
                for o in range(2):
                    u2 = ep_pool.tile([128, _EP], f32, tag="u2", name="u2")
                    # x = L1_o + d
                    nc.vector.tensor_tensor(
                        out=u2[P, 0:W], in0=Lbuf[P, ep0:ep0 + W, 2 + o],
                        in1=dv, op=Alu.add,
                    )
                    ex = ep_pool.tile([128, _EP], f32, tag="ex", name="ex")
                    nc.scalar.activation(ex[P, 0:W], u2[P, 0:W], Act.Exp)
                    # S = EL0_o + e^x
                    nc.vector.tensor_tensor(
                        out=u2[P, 0:W], in0=Lbuf[P, ep0:ep0 + W, o],
                        in1=ex[P, 0:W], op=Alu.add,
                    )
                    nc.scalar.activation(ex[P, 0:W], u2[P, 0:W], Act.Ln)
                    # lpy_o = ln(S) - ln(1+e^d)
                    nc.vector.tensor_tensor(
                        out=outbuf[P].rearrange("p (t o) -> p t o", o=2)
                            [:, ep0:ep0 + W, o],
                        in0=ex[P, 0:W], in1=zq, op=Alu.subtract,
                    )
                ep0 += W

        nc.sync.dma_start(out.ap(), outbuf[:])

    nc.compile()
    return nc


def _prep_inputs(corr, kc, problem, A, trans_logits, obs_logits_problem,
                 init_logits, T):
    corr = np.asarray(corr).astype(np.int64)
    kc = np.asarray(kc).astype(np.int64)
    problem = np.asarray(problem).astype(np.int64)
    taba, tabp, gam0, a3p0 = _host_tables(
        np.asarray(A), np.asarray(trans_logits),
        np.asarray(obs_logits_problem), np.asarray(init_logits))

    in_maps = []
    for i in range(N_CORES):
        sl = slice(i * BL, (i + 1) * BL)
        kc_l = kc[sl, :T]
        pp_l = 2 * problem[sl, :T] + corr[sl, :T]
        kcw = _wrap_idx(kc_l.T.ravel())
        ppw = _wrap_idx(pp_l.T.ravel())
        in_maps.append({
            "taba": taba, "tabp": tabp, "kcw": kcw, "ppw": ppw,
            "gam0": gam0, "a3p0": a3p0,
        })
    return in_maps


def kernel(corr, kc, problem, A, trans_logits, obs_logits_problem, init_logits,
           _T=None, _trace=False):
    T = _T or T_FULL
    nc = _emit_program(T, min(_CHUNK, T))
    in_maps = _prep_inputs(corr, kc, problem, A, trans_logits,
                           obs_logits_problem, init_logits, T)

    from concourse.bass_utils import run_bass_kernel_spmd
    res = run_bass_kernel_spmd(nc, in_maps, core_ids=list(range(N_CORES)),
                               trace=_trace)
    outs = [r["out"].reshape(BL, T, 2) for r in res.results]
    full = np.concatenate(outs, axis=0).astype(np.float32)
    kernel.last_results = res
    return full


if __name__ == "__main__":
    pass


# revision 3
# speedup vs baseline: 1.0423x; 1.0423x over previous
"""Trainium2 Bass kernel for the BKT (multi-HMM knowledge tracing) model.

Data-parallel over students (1024/8 = 128 per core, one SBUF partition each);
the T=500 time recursion runs locally per core.

Reformulated recursion (per student, per step t; state axis j, kept axis i):
    a2_j   = <c_t, alpha_j>                       c = A[kc[t]] (gathered)
    d      = a2_1 - a2_0
    a3_i   = a2_0 + M40_i + ln(1 + e^{dM_i + d})  (lse over j, shifted by j=0)
    alpha' = (1-c) * alpha + c * a3
    lpy_o  = ln(EL0_o + e^{L1_o + d}) - ln(1+e^d) (outputs depend only on d)

M4[i,j] = G4[i,j] + OLL_j mixes the kc-indexed G4 = A @ log_t with the
(problem,corr)-indexed OLL, so it is combined on-chip in two bulk ops per
8-step chunk (M4, then dM/M40 views); everything else is host-precomputed
into two gather tables.

State is kept as gamma = alpha - a3_prev, giving fused per-step updates
    a2_j   = <gamma_j + a3p_j, c>                 (one stt+accum; sum c = 1)
    gamma' = (gamma + (a3p - a3new)) * (1-c)      (one stt per state block)

The d / ln(1+e^d) streams and archived table columns feed a short bulk
epilogue that produces all T outputs (exp/ln outside the serial loop).

Per step: 7 DVE + 2 Act instructions, 2 cross-engine handoffs. Gathers run
on 2 SWDGE queues with a 64KB descriptor ring, 3 chunks ahead, with
independently-deep slab pools so the slab-reuse WAR never throttles
prefetch; the L-column archive copy rides the idle SP DMA queue.
"""

import os
from contextlib import ExitStack

import numpy as np

N_PROBLEMS = 10000
N_KCS = 100
BATCH = 1024
T_FULL = 500
N_CORES = 8
BL = BATCH // N_CORES  # 128 students per core

_CHUNK = 8  # time steps per gather slab (8*128 = 1024 = SWDGE ring capacity)
_EP = 125  # epilogue chunk width


def _log_softmax(x, axis):
    x = x.astype(np.float32)
    m = x.max(axis=axis, keepdims=True)
    e = np.exp(x - m)
    return (x - m) - np.log(e.sum(axis=axis, keepdims=True))


def _wrap_idx(flat):
    """dma_gather index layout: flat index i lives at partition i%16, col i//16,
    replicated across the 8 gpsimd cores (16-partition groups)."""
    assert flat.size % 16 == 0
    w = flat.astype(np.int16).reshape(-1, 16).T  # [16, N/16]
    return np.tile(w, (8, 1))  # [128, N/16]


def _host_tables(A, trans_logits, obs_logits_problem, init_logits):
    P = A.shape[0]
    K = trans_logits.shape[0]
    log_t = _log_softmax(trans_logits, axis=1)  # [K,2,2] normalized over dim 1
    G = A.astype(np.float32) @ log_t.reshape(K, 4)  # [P,4] pairs (i,j)
    L = _log_softmax(obs_logits_problem, axis=2)  # [P,2,2] normalized over o

    # taba (kc-indexed), f32 1KB rows:
    # cols [0:100] = A, [100:104] = G4 (i,j) pairs, [128:228] = 1 - A
    taba = np.zeros((P, 256), np.float32)
    taba[:, 0:100] = A.astype(np.float32)
    taba[:, 100:104] = G
    taba[:, 128:228] = (1.0 - A).astype(np.float32)

    # tabp row (2*problem + corr), f32 256B rows:
    #  [0]=OLL_0 [1]=OLL_1 | [2:4]=EL0_o | [4:6]=L1_o
    tabp = np.zeros((2 * P, 64), np.float32)
    for corr in (0, 1):
        tabp[corr::2, 0] = L[:, 0, corr]
        tabp[corr::2, 1] = L[:, 1, corr]
    tabp[:, 2:4] = np.exp(np.repeat(L[:, 0, :], 2, axis=0))  # EL0_o
    tabp[:, 4:6] = np.repeat(L[:, 1, :], 2, axis=0)          # L1_o

    la0 = _log_softmax(init_logits, axis=1)  # [K,2]
    g0 = np.empty((BL, 2 * K + 2), np.float32)
    g0[:, 0:K] = la0[:, 0]  # state block j=0
    g0[:, K:2 * K] = la0[:, 1]  # j=1
    g0[:, 2 * K:] = 0.0  # a3p seed zeros
    return taba, tabp, g0, g0[:, 2 * K:].copy()


def _setup_act_tables():
    """Make bacc+walrus agree that Exp and Ln share one ACT table set, so the
    kernel does a single ACT_TABLE_LOAD instead of one per activation."""
    import glob
    import json
    import tempfile

    if os.environ.get("_BKT_ACT_TABLES"):
        return
    from neuronxcc.driver.Job import Job  # pyright: ignore[reportMissingImports]
    from neuronxcc.driver.jobs.support.FindActInfo import (  # pyright: ignore[reportMissingImports]
        findActInfoFile,
    )

    src = findActInfoFile(Job.getPackageDir(), "gen3")
    d = json.load(open(src))
    d["act_func_sets"] = sorted(
        d["act_func_sets"],
        key=lambda s: s["name"] != "natural_log_exp_and_others")
    tmp = tempfile.mkdtemp(prefix="bkt_act_")
    with open(tmp + "/act_info.json", "w") as f:
        json.dump(d, f)
    for p in glob.glob(os.path.dirname(src) + "/*"):
        b = os.path.basename(p)
        if b != "act_info.json":
            os.symlink(p, tmp + "/" + b)
    os.environ["BASS_ACT_ROOT_JSON_PATH"] = tmp + "/act_info.json"
    os.environ["_BKT_ACT_TABLES"] = "1"

    import concourse.bacc as bacc_mod
    import concourse.mybir as mybir

    def tables(arch):
        out = {
            e["name"]: {mybir.ActivationFunctionType.from_pwp(v)
                        for v in e["act"].keys()}
            for e in d["act_func_sets"]
        }
        # act_info.json lists softplus as the generic 'act2' slot; teach the
        # bass-side chooser that this set implements Softplus.
        out["softplus_and_others"].add(mybir.ActivationFunctionType.Softplus)
        return out

    bacc_mod.get_activation_tables = tables


def _emit_program(T, Tc):
    import concourse.mybir as mybir
    import concourse.tile as tile
    from concourse import bacc

    _setup_act_tables()

    f32 = mybir.dt.float32
    f16 = mybir.dt.float16
    i16 = mybir.dt.int16
    Alu = mybir.AluOpType
    Act = mybir.ActivationFunctionType
    K = N_KCS

    nc = bacc.Bacc("TRN2", target_bir_lowering=False, debug=False,
                   dynamic_dma_scratch_size=65536, num_swdge_queues=2,
                   monotonic_sem_count=int(os.environ.get("_NSEM", "1")))

    taba = nc.dram_tensor("taba", [N_PROBLEMS, 256], f32, kind="ExternalInput")
    tabp = nc.dram_tensor("tabp", [2 * N_PROBLEMS, 64], f32, kind="ExternalInput")
    kcw = nc.dram_tensor("kcw", [128, T * 8], i16, kind="ExternalInput")
    ppw = nc.dram_tensor("ppw", [128, T * 8], i16, kind="ExternalInput")
    gam0 = nc.dram_tensor("gam0", [BL, 2 * K + 2], f32, kind="ExternalInput")
    a3p0 = nc.dram_tensor("a3p0", [BL, 2], f32, kind="ExternalInput")
    out = nc.dram_tensor("out", [BL, T * 2], f32, kind="ExternalOutput")

    assert Tc * 128 <= 1024  # SWDGE descriptor ring capacity per gather
    chunks = []
    t0 = 0
    while t0 < T:
        chunks.append((t0, min(Tc, T - t0)))
        t0 += Tc
    n_chunks = len(chunks)

    from concourse import library_config

    with ExitStack() as ctx:
        tc = ctx.enter_context(tile.TileContext(nc))
        nc.gpsimd.load_library(library_config.mlp)
        idx_pool = ctx.enter_context(tc.tile_pool(name="idx", bufs=1))
        slab_pool = ctx.enter_context(tc.tile_pool(name="slabs", bufs=int(os.environ.get("_BA", "4"))))
        slabp_pool = ctx.enter_context(tc.tile_pool(name="slabsp", bufs=int(os.environ.get("_BP", "8"))))
        m4_pool = ctx.enter_context(tc.tile_pool(name="m4p", bufs=3))
        state_pool = ctx.enter_context(tc.tile_pool(name="state", bufs=2))
        small_pool = ctx.enter_context(tc.tile_pool(name="small", bufs=2))
        big_pool = ctx.enter_context(tc.tile_pool(name="big", bufs=1))
        ep_pool = ctx.enter_context(tc.tile_pool(name="ep", bufs=2))

        kcw_t = idx_pool.tile([128, T * 8], i16, tag="kcw")
        nc.sync.dma_start(kcw_t[:], kcw.ap())
        ppw_t = idx_pool.tile([128, T * 8], i16, tag="ppw")
        nc.sync.dma_start(ppw_t[:], ppw.ap())

        # Per-chain state/scratch (chains: partitions 0:64 and 64:128).
        # Separate tiles per chain so the dep tracker never sees a
        # same-tile write-write conflict between the two chains.
        gamma, a3p, dbuf, l2buf, a2p, scr, e2 = {}, {}, {}, {}, {}, {}, {}
        for ci in range(2):
            gamma[ci] = [state_pool.tile([128, 2 * K], f32,
                                         tag=f"gam{ci}_{i}", name=f"gam{ci}_{i}")
                         for i in range(2)]
            a3p[ci] = [state_pool.tile([128, 2], f32, tag=f"a3p{ci}_{i}",
                                       name=f"a3p{ci}_{i}") for i in range(2)]
            dbuf[ci] = big_pool.tile([128, T], f32, tag=f"dbuf{ci}",
                                     name=f"dbuf{ci}")
            l2buf[ci] = big_pool.tile([128, 2 * T], f32, tag=f"l2buf{ci}",
                                      name=f"l2buf{ci}")
            a2p[ci] = big_pool.tile([128, 2], f32, tag=f"a2p{ci}",
                                    name=f"a2p{ci}")
            scr[ci] = big_pool.tile([128, K], f32, tag=f"scr{ci}",
                                    name=f"scr{ci}")
            e2[ci] = big_pool.tile([128, 2], f32, tag=f"e2{ci}",
                                   name=f"e2{ci}")
        NCH = int(os.environ.get("_NCHAINS", "1"))
        P_OF = {0: slice(0, 64), 1: slice(64, 128)} if NCH == 2 else {0: slice(0, 128)}
        for ci in P_OF:
            P = P_OF[ci]
            nc.sync.dma_start(gamma[ci][0][P], gam0.ap()[P, 0:2 * K])
            nc.sync.dma_start(a3p[ci][0][P], a3p0.ap()[P, :])
        Lbuf = big_pool.tile([128, T, 4], f32, tag="Lbuf")
        outbuf = big_pool.tile([128, T * 2], f32, tag="outbuf")

        slabsA = [None] * n_chunks
        slabsP = [None] * n_chunks
        m4s = [None] * n_chunks
        dms = [None] * n_chunks
        ni_regs = {}
        for tcn in sorted({c[1] for c in chunks}):
            r = nc.gpsimd.alloc_register(f"ni{tcn}")
            nc.gpsimd.reg_mov(r, tcn * 128)
            ni_regs[tcn] = r

        def issue_gather(n):
            t0, tcn = chunks[n]
            ni = ni_regs[tcn]
            sa = slab_pool.tile([128, Tc, 256], f32, tag="slabA")
            nc.gpsimd.dma_gather(
                sa[:, 0:tcn, :], taba.ap(), kcw_t[:, t0 * 8:(t0 + tcn) * 8],
                num_idxs=tcn * 128, num_idxs_reg=ni, elem_size=256,
                queue_num=0,
            )
            sp = slabp_pool.tile([128, Tc, 64], f32, tag="slabP")
            nc.gpsimd.dma_gather(
                sp[:, 0:tcn, :], tabp.ap(), ppw_t[:, t0 * 8:(t0 + tcn) * 8],
                num_idxs=tcn * 128, num_idxs_reg=ni, elem_size=64,
                queue_num=1,
            )
            slabsA[n], slabsP[n] = sa, sp

        def issue_m4(n):
            t0, tcn = chunks[n]
            sa, sp = slabsA[n], slabsP[n]
            m4 = m4_pool.tile([128, Tc, 4], f32, tag="m4", name="m4")
            # M4[i,j] = G4[i,j] + OLL_j  (broadcast over i)
            nc.vector.tensor_tensor(
                out=m4[:, 0:tcn, :].rearrange("p t (i j) -> p t i j", j=2),
                in0=sa[:, 0:tcn, 100:104].rearrange("p t (i j) -> p t i j", j=2),
                in1=sp[:, 0:tcn, 0:2].unsqueeze(2).broadcast_to(
                    [128, tcn, 2, 2]),
                op=Alu.add,
            )
            dm = m4_pool.tile([128, Tc, 2], f32, tag="dm", name="dm")
            m4v = m4[:, 0:tcn, :].rearrange("p t (i j) -> p t i j", j=2)
            # dM_i = M4[i,1] - M4[i,0]
            nc.vector.tensor_tensor(
                out=dm[:, 0:tcn, :], in0=m4v[:, :, :, 1], in1=m4v[:, :, :, 0],
                op=Alu.subtract,
            )
            m4s[n], dms[n] = m4, dm

        issue_gather(0)
        issue_gather(1)
        issue_gather(2)
        issue_m4(0)

        for n in range(n_chunks):
            t0, tcn = chunks[n]
            if n + 3 < n_chunks:
                issue_gather(n + 3)
            sa, sp = slabsA[n], slabsP[n]
            m4, dm = m4s[n], dms[n]
            m4v = m4[:, 0:Tc, :].rearrange("p t (i j) -> p t i j", j=2)
            # Two independent 64-partition chains (students 0-63 / 64-127)
            # interleaved so each chain's compute hides the other's
            # dependency stalls. Chains have private tiles; slabs are shared
            # read-only.
            def front(t, j, ci):
                """a2 pair + d for chain ci."""
                P = P_OF[ci]
                cur = a3p[ci][t % 2][P]
                gcur = gamma[ci][t % 2][P]
                c_ap = sa[P, j, 0:K]
                for s in range(2):
                    nc.vector.scalar_tensor_tensor(
                        out=scr[ci][P], in0=gcur[:, s * K:(s + 1) * K],
                        scalar=cur[:, s:s + 1], in1=c_ap,
                        op0=Alu.add, op1=Alu.mult,
                        accum_out=a2p[ci][P, s:s + 1],
                    )
                # d = a2_1 - a2_0
                nc.vector.tensor_tensor(
                    out=dbuf[ci][P, t:t + 1], in0=a2p[ci][P, 1:2],
                    in1=a2p[ci][P, 0:1], op=Alu.subtract,
                )

            def acts(t, j, ci):
                P = P_OF[ci]
                # e2 = exp([dM_0, dM_1] + d)
                nc.scalar.activation(
                    e2[ci][P], dm[P, j, 0:2], Act.Exp,
                    bias=dbuf[ci][P, t:t + 1],
                )
                # l2 = ln(e2 + 1) -> ln1p pair
                nc.scalar.activation(l2buf[ci][P, 2 * t:2 * t + 2],
                                     e2[ci][P], Act.Ln, bias=1.0)

            def back(t, j, ci, b2):
                P = P_OF[ci]
                cur = a3p[ci][t % 2][P]
                nxt = a3p[ci][(t + 1) % 2][P]
                gcur = gamma[ci][t % 2][P]
                gnxt = gamma[ci][(t + 1) % 2][P]
                cm_ap = sa[P, j, 128:128 + K]
                lg = l2buf[ci][P, 2 * t:2 * t + 2]
                # a3new_i = (ln1p_i + a2_0) + M40_i
                nc.vector.scalar_tensor_tensor(
                    out=nxt[:], in0=lg[:, 0:2], scalar=a2p[ci][P, 0:1],
                    in1=m4v[P, j, :, 0], op0=Alu.add, op1=Alu.add,
                )
                # b2 = a3p - a3new
                nc.vector.tensor_tensor(
                    out=b2[P], in0=cur[:], in1=nxt[:], op=Alu.subtract,
                )
                # gamma'_s = (gamma_s + b2_s) * (1 - c)
                for s in range(2):
                    nc.vector.scalar_tensor_tensor(
                        out=gnxt[:, s * K:(s + 1) * K],
                        in0=gcur[:, s * K:(s + 1) * K],
                        scalar=b2[P, s:s + 1], in1=cm_ap,
                        op0=Alu.add, op1=Alu.mult,
                    )

            for j in range(tcn):
                t = t0 + j
                b2a = small_pool.tile([128, 2], f32, tag="b2a", name="b2a")
                if NCH == 2:
                    b2b = small_pool.tile([128, 2], f32, tag="b2b", name="b2b")
                    front(t, j, 0)
                    acts(t, j, 0)
                    front(t, j, 1)
                    acts(t, j, 1)
                    back(t, j, 0, b2a)
                    back(t, j, 1, b2b)
                else:
                    front(t, j, 0)
                    acts(t, j, 0)
                    back(t, j, 0, b2a)
            if n + 1 < n_chunks:
                issue_m4(n + 1)
            # archive the epilogue columns [EL0_0, EL0_1, L1_0, L1_1] via an
            # SBUF->SBUF DMA on the idle SP queue (keeps all compute engines
            # out of the slab-lifetime dependency chain)
            nc.sync.dma_start(Lbuf[:, t0:t0 + tcn, :], sp[:, 0:tcn, 2:6])

        # ---- epilogue: log_py from d stream + archived L columns ----
        for ci in P_OF:
            P = P_OF[ci]
            ep0 = 0
            while ep0 < T:
                W = min(_EP, T - ep0)
                dv = dbuf[ci][P, ep0:ep0 + W]
                # zq = ln(1 + e^d)
                zqe = ep_pool.tile([128, _EP], f32, tag="zqe", name="zqe")
                nc.scalar.activation(zqe[P, 0:W], dv, Act.Exp)
                zqt = ep_pool.tile([128, _EP], f32, tag="zqt", name="zqt")
                nc.scalar.activation(zqt[P, 0:W], zqe[P, 0:W], Act.Ln,
                                     bias=1.0)
                zq = zqt[P, 0:W]
                for o in range(2):
                    u2 = ep_pool.tile([128, _EP], f32, tag="u2", name="u2")
                    # x = L1_o + d
                    nc.vector.tensor_tensor(
                        out=u2[P, 0:W], in0=Lbuf[P, ep0:ep0 + W, 2 + o],
                        in1=dv, op=Alu.add,
                    )
                    ex = ep_pool.tile([128, _EP], f32, tag="ex", name="ex")
                    nc.scalar.activation(ex[P, 0:W], u2[P, 0:W], Act.Exp)
                    # S = EL0_o + e^x
                    nc.vector.tensor_tensor(
                        out=u2[P, 0:W], in0=Lbuf[P, ep0:ep0 + W, o],
                        in1=ex[P, 0:W], op=Alu.add,
                    )
                    nc.scalar.activation(ex[P, 0:W], u2[P, 0:W], Act.Ln)
                    # lpy_o = ln(S) - ln(1+e^d)
                    nc.vector.tensor_tensor(
                        out=outbuf[P].rearrange("p (t o) -> p t o", o=2)
                            [:, ep0:ep0 + W, o],
                        in0=ex[P, 0:W], in1=zq, op=Alu.subtract,
                    )
                ep0 += W

        nc.sync.dma_start(out.ap(), outbuf[:])

    nc.compile()
    return nc


def _prep_inputs(corr, kc, problem, A, trans_logits, obs_logits_problem,
                 init_logits, T):
    corr = np.asarray(corr).astype(np.int64)
    kc = np.asarray(kc).astype(np.int64)
    problem = np.asarray(problem).astype(np.int64)
    taba, tabp, gam0, a3p0 = _host_tables(
        np.asarray(A), np.asarray(trans_logits),
        np.asarray(obs_logits_problem), np.asarray(init_logits))

    in_maps = []
    for i in range(N_CORES):
        sl = slice(i * BL, (i + 1) * BL)
        kc_l = kc[sl, :T]
        pp_l = 2 * problem[sl, :T] + corr[sl, :T]
        kcw = _wrap_idx(kc_l.T.ravel())
        ppw = _wrap_idx(pp_l.T.ravel())
        in_maps.append({
            "taba": taba, "tabp": tabp, "kcw": kcw, "ppw": ppw,
            "gam0": gam0, "a3p0": a3p0,
        })
    return in_maps


def kernel(corr, kc, problem, A, trans_logits, obs_logits_problem, init_logits,
           _T=None, _trace=False):
    T = _T or T_FULL
    nc = _emit_program(T, min(_CHUNK, T))
    in_maps = _prep_inputs(corr, kc, problem, A, trans_logits,
                           obs_logits_problem, init_logits, T)

    from concourse.bass_utils import run_bass_kernel_spmd
    res = run_bass_kernel_spmd(nc, in_maps, core_ids=list(range(N_CORES)),
                               trace=_trace)
    outs = [r["out"].reshape(BL, T, 2) for r in res.results]
    full = np.concatenate(outs, axis=0).astype(np.float32)
    kernel.last_results = res
    return full


if __name__ == "__main__":
    pass


# revision 5
# speedup vs baseline: 1.0830x; 1.0390x over previous
"""Trainium2 Bass kernel v2 for the BKT (multi-HMM knowledge tracing) model.

Data-parallel over students (1024/8 = 128 per core, one SBUF partition each);
the T=500 recursion runs locally per core.

Reformulated recursion (per student, per step t):
    a2_j   = <c_t, alpha_j>                       j in {0,1} (state axis)
    d      = a2_1 - a2_0
    a3_i   = a2_0 + M40_i + ln(1 + e^{dM_i + d})  (lse over j, shifted by j=0)
    alpha' = (1-c) * alpha + c * a3               (per state block)
    lpy_o  = ln(EL0_o + e^{L1_o + d}) - ln(1+e^d) (outputs; depend only on d!)

with host-precomputed gather tables:
    M40_i = G4[i,0]+OLL_0, dM_i = G4[i,1]-G4[i,0]+OLL_1-OLL_0,
    G4 = A @ log_t, OLL_j = log_obs[p, j, corr], EL0_o = e^{L[p,0,o]},
    L1_o = L[p,1,o].

State kept as gamma = alpha - a3_prev (fp16), giving the fused updates
    a2_j    = a3p_j + <c, gamma_j>                (one ttr per j, seed=a3p_j)
    gamma'_ = ((gamma + (a3p - a3new)) * (1-c))   (one stt per state block)

Outputs are NOT computed in the loop: d and ln(1+e^d) stream to SBUF and a
short bulk epilogue (exp/ln over the archived L-columns) produces log_py.

Per step: 7 DVE + 2 Act instructions, 2 cross-engine handoffs.
"""

import os
from contextlib import ExitStack

import numpy as np

N_PROBLEMS = 10000
N_KCS = 100
BATCH = 1024
T_FULL = 500
N_CORES = 8
BL = BATCH // N_CORES  # 128 students per core

_CHUNK = 8  # time steps per gather slab (8*128 = 1024 = SWDGE ring capacity)
_EP = 125  # epilogue chunk width


def _log_softmax(x, axis):
    x = x.astype(np.float32)
    m = x.max(axis=axis, keepdims=True)
    e = np.exp(x - m)
    return (x - m) - np.log(e.sum(axis=axis, keepdims=True))


def _wrap_idx(flat):
    """dma_gather index layout: flat index i lives at partition i%16, col i//16,
    replicated across the 8 gpsimd cores (16-partition groups)."""
    assert flat.size % 16 == 0
    w = flat.astype(np.int16).reshape(-1, 16).T  # [16, N/16]
    return np.tile(w, (8, 1))  # [128, N/16]


def _host_tables(A, trans_logits, obs_logits_problem, init_logits):
    P = A.shape[0]
    K = trans_logits.shape[0]
    log_t = _log_softmax(trans_logits, axis=1)  # [K,2,2] normalized over dim 1
    G = A.astype(np.float32) @ log_t.reshape(K, 4)  # [P,4] pairs (i,j)
    L = _log_softmax(obs_logits_problem, axis=2)  # [P,2,2] normalized over o

    # taba (kc-indexed), f32 1KB rows:
    # cols [0:100] = A, [100:104] = G4 (i,j) pairs, [128:228] = 1 - A
    taba = np.zeros((P, 256), np.float32)
    taba[:, 0:100] = A.astype(np.float32)
    taba[:, 100:104] = G
    taba[:, 128:228] = (1.0 - A).astype(np.float32)

    # tabp row (2*problem + corr), f32 256B rows:
    #  [0]=OLL_0 [1]=OLL_1 | [2:4]=EL0_o | [4:6]=L1_o
    tabp = np.zeros((2 * P, 64), np.float32)
    for corr in (0, 1):
        tabp[corr::2, 0] = L[:, 0, corr]
        tabp[corr::2, 1] = L[:, 1, corr]
    tabp[:, 2:4] = np.exp(np.repeat(L[:, 0, :], 2, axis=0))  # EL0_o
    tabp[:, 4:6] = np.repeat(L[:, 1, :], 2, axis=0)          # L1_o

    la0 = _log_softmax(init_logits, axis=1)  # [K,2]
    g0 = np.empty((BL, 2 * K + 2), np.float32)
    g0[:, 0:K] = la0[:, 0]  # state block j=0
    g0[:, K:2 * K] = la0[:, 1]  # j=1
    g0[:, 2 * K:] = 0.0  # a3p seed zeros
    return taba, tabp, g0, g0[:, 2 * K:].copy()


def _setup_act_tables():
    """Make bacc+walrus agree that Exp and Ln share one ACT table set, so the
    kernel does a single ACT_TABLE_LOAD instead of one per activation."""
    import glob
    import json
    import tempfile

    if os.environ.get("_BKT_ACT_TABLES"):
        return
    from neuronxcc.driver.Job import Job  # pyright: ignore[reportMissingImports]
    from neuronxcc.driver.jobs.support.FindActInfo import (  # pyright: ignore[reportMissingImports]
        findActInfoFile,
    )

    src = findActInfoFile(Job.getPackageDir(), "gen3")
    d = json.load(open(src))
    d["act_func_sets"] = sorted(
        d["act_func_sets"],
        key=lambda s: s["name"] != "natural_log_exp_and_others")
    tmp = tempfile.mkdtemp(prefix="bkt_act_")
    with open(tmp + "/act_info.json", "w") as f:
        json.dump(d, f)
    for p in glob.glob(os.path.dirname(src) + "/*"):
        b = os.path.basename(p)
        if b != "act_info.json":
            os.symlink(p, tmp + "/" + b)
    os.environ["BASS_ACT_ROOT_JSON_PATH"] = tmp + "/act_info.json"
    os.environ["_BKT_ACT_TABLES"] = "1"

    import concourse.bacc as bacc_mod
    import concourse.mybir as mybir

    def tables(arch):
        out = {
            e["name"]: {mybir.ActivationFunctionType.from_pwp(v)
                        for v in e["act"].keys()}
            for e in d["act_func_sets"]
        }
        # act_info.json lists softplus as the generic 'act2' slot; teach the
        # bass-side chooser that this set implements Softplus.
        out["softplus_and_others"].add(mybir.ActivationFunctionType.Softplus)
        return out

    bacc_mod.get_activation_tables = tables


def _emit_program(T, Tc):
    import concourse.mybir as mybir
    import concourse.tile as tile
    from concourse import bacc

    _setup_act_tables()

    f32 = mybir.dt.float32
    f16 = mybir.dt.float16
    i16 = mybir.dt.int16
    Alu = mybir.AluOpType
    Act = mybir.ActivationFunctionType
    K = N_KCS

    nc = bacc.Bacc("TRN2", target_bir_lowering=False, debug=False,
                   dynamic_dma_scratch_size=65536, num_swdge_queues=2,
                   monotonic_sem_count=int(os.environ.get("_NSEM", "1")))

    taba = nc.dram_tensor("taba", [N_PROBLEMS, 256], f32, kind="ExternalInput")
    tabp = nc.dram_tensor("tabp", [2 * N_PROBLEMS, 64], f32, kind="ExternalInput")
    kcw = nc.dram_tensor("kcw", [128, T * 8], i16, kind="ExternalInput")
    ppw = nc.dram_tensor("ppw", [128, T * 8], i16, kind="ExternalInput")
    gam0 = nc.dram_tensor("gam0", [BL, 2 * K + 2], f32, kind="ExternalInput")
    a3p0 = nc.dram_tensor("a3p0", [BL, 2], f32, kind="ExternalInput")
    out = nc.dram_tensor("out", [BL, T * 2], f32, kind="ExternalOutput")

    assert Tc * 128 <= 1024  # SWDGE descriptor ring capacity per gather
    chunks = []
    t0 = 0
    while t0 < T:
        chunks.append((t0, min(Tc, T - t0)))
        t0 += Tc
    n_chunks = len(chunks)

    from concourse import library_config

    with ExitStack() as ctx:
        tc = ctx.enter_context(tile.TileContext(nc))
        nc.gpsimd.load_library(library_config.mlp)
        idx_pool = ctx.enter_context(tc.tile_pool(name="idx", bufs=1))
        slab_pool = ctx.enter_context(tc.tile_pool(name="slabs", bufs=int(os.environ.get("_BA", "4"))))
        slabp_pool = ctx.enter_context(tc.tile_pool(name="slabsp", bufs=int(os.environ.get("_BP", "8"))))
        m4_pool = ctx.enter_context(tc.tile_pool(name="m4p", bufs=3))
        state_pool = ctx.enter_context(tc.tile_pool(name="state", bufs=2))
        small_pool = ctx.enter_context(tc.tile_pool(name="small", bufs=2))
        big_pool = ctx.enter_context(tc.tile_pool(name="big", bufs=1))
        ep_pool = ctx.enter_context(tc.tile_pool(name="ep", bufs=2))

        kcw_t = idx_pool.tile([128, T * 8], i16, tag="kcw")
        nc.sync.dma_start(kcw_t[:], kcw.ap())
        ppw_t = idx_pool.tile([128, T * 8], i16, tag="ppw")
        nc.sync.dma_start(ppw_t[:], ppw.ap())

        # Per-chain state/scratch (chains: partitions 0:64 and 64:128).
        # Separate tiles per chain so the dep tracker never sees a
        # same-tile write-write conflict between the two chains.
        gamma, a3p, dbuf, l2buf, a2p, scr, e2 = {}, {}, {}, {}, {}, {}, {}
        for ci in range(2):
            gamma[ci] = [state_pool.tile([128, 2 * K], f32,
                                         tag=f"gam{ci}_{i}", name=f"gam{ci}_{i}")
                         for i in range(2)]
            a3p[ci] = [state_pool.tile([128, 2], f32, tag=f"a3p{ci}_{i}",
                                       name=f"a3p{ci}_{i}") for i in range(2)]
            dbuf[ci] = big_pool.tile([128, T], f32, tag=f"dbuf{ci}",
                                     name=f"dbuf{ci}")
            l2buf[ci] = big_pool.tile([128, 2 * T], f32, tag=f"l2buf{ci}",
                                      name=f"l2buf{ci}")
            a2p[ci] = big_pool.tile([128, 2], f32, tag=f"a2p{ci}",
                                    name=f"a2p{ci}")
            scr[ci] = big_pool.tile([128, K], f32, tag=f"scr{ci}",
                                    name=f"scr{ci}")
            e2[ci] = [big_pool.tile([128, 2], f32, tag=f"e2{ci}_{i}",
                                    name=f"e2{ci}_{i}") for i in range(2)]
        NCH = int(os.environ.get("_NCHAINS", "1"))
        P_OF = {0: slice(0, 64), 1: slice(64, 128)} if NCH == 2 else {0: slice(0, 128)}
        for ci in P_OF:
            P = P_OF[ci]
            nc.sync.dma_start(gamma[ci][0][P], gam0.ap()[P, 0:2 * K])
            nc.sync.dma_start(a3p[ci][0][P], a3p0.ap()[P, :])
        Lbuf = big_pool.tile([128, T, 4], f32, tag="Lbuf")
        outbuf = big_pool.tile([128, T * 2], f32, tag="outbuf")

        slabsA = [None] * n_chunks
        slabsP = [None] * n_chunks
        m4s = [None] * n_chunks
        dms = [None] * n_chunks
        ni_regs = {}
        for tcn in sorted({c[1] for c in chunks}):
            r = nc.gpsimd.alloc_register(f"ni{tcn}")
            nc.gpsimd.reg_mov(r, tcn * 128)
            ni_regs[tcn] = r

        def issue_gather(n):
            t0, tcn = chunks[n]
            ni = ni_regs[tcn]
            sa = slab_pool.tile([128, Tc, 256], f32, tag="slabA")
            nc.gpsimd.dma_gather(
                sa[:, 0:tcn, :], taba.ap(), kcw_t[:, t0 * 8:(t0 + tcn) * 8],
                num_idxs=tcn * 128, num_idxs_reg=ni, elem_size=256,
                queue_num=0,
            )
            sp = slabp_pool.tile([128, Tc, 64], f32, tag="slabP")
            nc.gpsimd.dma_gather(
                sp[:, 0:tcn, :], tabp.ap(), ppw_t[:, t0 * 8:(t0 + tcn) * 8],
                num_idxs=tcn * 128, num_idxs_reg=ni, elem_size=64,
                queue_num=1,
            )
            slabsA[n], slabsP[n] = sa, sp

        def issue_m4(n):
            t0, tcn = chunks[n]
            sa, sp = slabsA[n], slabsP[n]
            m4 = m4_pool.tile([128, Tc, 4], f32, tag="m4", name="m4")
            # M4[i,j] = G4[i,j] + OLL_j  (broadcast over i)
            nc.vector.tensor_tensor(
                out=m4[:, 0:tcn, :].rearrange("p t (i j) -> p t i j", j=2),
                in0=sa[:, 0:tcn, 100:104].rearrange("p t (i j) -> p t i j", j=2),
                in1=sp[:, 0:tcn, 0:2].unsqueeze(2).broadcast_to(
                    [128, tcn, 2, 2]),
                op=Alu.add,
            )
            dm = m4_pool.tile([128, Tc, 2], f32, tag="dm", name="dm")
            m4v = m4[:, 0:tcn, :].rearrange("p t (i j) -> p t i j", j=2)
            # dM_i = M4[i,1] - M4[i,0]
            nc.vector.tensor_tensor(
                out=dm[:, 0:tcn, :], in0=m4v[:, :, :, 1], in1=m4v[:, :, :, 0],
                op=Alu.subtract,
            )
            m4s[n], dms[n] = m4, dm

        issue_gather(0)
        issue_gather(1)
        issue_gather(2)
        issue_m4(0)

        for n in range(n_chunks):
            t0, tcn = chunks[n]
            if n + 3 < n_chunks:
                issue_gather(n + 3)
            sa, sp = slabsA[n], slabsP[n]
            m4, dm = m4s[n], dms[n]
            m4v = m4[:, 0:Tc, :].rearrange("p t (i j) -> p t i j", j=2)
            # Two independent 64-partition chains (students 0-63 / 64-127)
            # interleaved so each chain's compute hides the other's
            # dependency stalls. Chains have private tiles; slabs are shared
            # read-only.
            def front(t, j, ci):
                """a2 pair + d for chain ci."""
                P = P_OF[ci]
                cur = a3p[ci][t % 2][P]
                gcur = gamma[ci][t % 2][P]
                c_ap = sa[P, j, 0:K]
                for s in range(2):
                    nc.vector.scalar_tensor_tensor(
                        out=scr[ci][P], in0=gcur[:, s * K:(s + 1) * K],
                        scalar=cur[:, s:s + 1], in1=c_ap,
                        op0=Alu.add, op1=Alu.mult,
                        accum_out=a2p[ci][P, s:s + 1],
                    )
                # d = a2_1 - a2_0
                nc.vector.tensor_tensor(
                    out=dbuf[ci][P, t:t + 1], in0=a2p[ci][P, 1:2],
                    in1=a2p[ci][P, 0:1], op=Alu.subtract,
                )

            def acts(t, j, ci):
                P = P_OF[ci]
                # e2 = exp([dM_0, dM_1] + d)  (ping-pong: no cross-step WAR)
                e2t = e2[ci][t % 2]
                nc.scalar.activation(
                    e2t[P], dm[P, j, 0:2], Act.Exp,
                    bias=dbuf[ci][P, t:t + 1],
                )
                # l2 = ln(e2 + 1) -> ln1p pair
                nc.scalar.activation(l2buf[ci][P, 2 * t:2 * t + 2],
                                     e2t[P], Act.Ln, bias=1.0)

            def back(t, j, ci, b2):
                P = P_OF[ci]
                cur = a3p[ci][t % 2][P]
                nxt = a3p[ci][(t + 1) % 2][P]
                gcur = gamma[ci][t % 2][P]
                gnxt = gamma[ci][(t + 1) % 2][P]
                cm_ap = sa[P, j, 128:128 + K]
                lg = l2buf[ci][P, 2 * t:2 * t + 2]
                # cm40 = a3p - M40 runs while Act computes Exp/Ln, so the
                # post-Ln chain is one hop shorter:
                #   nb2 = (ln1p + a2_0) - cm40   (= a3new - a3p = -b2)
                cm40 = small_pool.tile([128, 2], f32, tag=f"cm40{ci}",
                                       name="cm40")
                nc.vector.tensor_tensor(
                    out=cm40[P], in0=cur[:], in1=m4v[P, j, :, 0],
                    op=Alu.subtract,
                )
                nc.vector.scalar_tensor_tensor(
                    out=b2[P], in0=lg[:, 0:2], scalar=a2p[ci][P, 0:1],
                    in1=cm40[P], op0=Alu.add, op1=Alu.subtract,
                )
                # a3p_next = a3p + nb2 (off the immediate chain)
                nc.vector.tensor_tensor(
                    out=nxt[:], in0=cur[:], in1=b2[P], op=Alu.add,
                )
                # gamma'_s = (gamma_s - nb2_s) * (1 - c)
                for s in range(2):
                    nc.vector.scalar_tensor_tensor(
                        out=gnxt[:, s * K:(s + 1) * K],
                        in0=gcur[:, s * K:(s + 1) * K],
                        scalar=b2[P, s:s + 1], in1=cm_ap,
                        op0=Alu.subtract, op1=Alu.mult,
                    )

            for j in range(tcn):
                t = t0 + j
                b2a = small_pool.tile([128, 2], f32, tag="b2a", name="b2a")
                if NCH == 2:
                    b2b = small_pool.tile([128, 2], f32, tag="b2b", name="b2b")
                    front(t, j, 0)
                    acts(t, j, 0)
                    front(t, j, 1)
                    acts(t, j, 1)
                    back(t, j, 0, b2a)
                    back(t, j, 1, b2b)
                else:
                    front(t, j, 0)
                    acts(t, j, 0)
                    back(t, j, 0, b2a)
            if n + 1 < n_chunks:
                issue_m4(n + 1)
            # archive the epilogue columns [EL0_0, EL0_1, L1_0, L1_1] via an
            # SBUF->SBUF DMA on the idle SP queue (keeps all compute engines
            # out of the slab-lifetime dependency chain)
            nc.sync.dma_start(Lbuf[:, t0:t0 + tcn, :], sp[:, 0:tcn, 2:6])

        # ---- epilogue: log_py from d stream + archived L columns ----
        for ci in P_OF:
            P = P_OF[ci]
            ep0 = 0
            while ep0 < T:
                W = min(_EP, T - ep0)
                dv = dbuf[ci][P, ep0:ep0 + W]
                # zq = ln(1 + e^d)
                zqe = ep_pool.tile([128, _EP], f32, tag="zqe", name="zqe")
                nc.scalar.activation(zqe[P, 0:W], dv, Act.Exp)
                zqt = ep_pool.tile([128, _EP], f32, tag="zqt", name="zqt")
                nc.scalar.activation(zqt[P, 0:W], zqe[P, 0:W], Act.Ln,
                                     bias=1.0)
                zq = zqt[P, 0:W]
                for o in range(2):
                    u2 = ep_pool.tile([128, _EP], f32, tag="u2", name="u2")
                    # x = L1_o + d
                    nc.vector.tensor_tensor(
                        out=u2[P, 0:W], in0=Lbuf[P, ep0:ep0 + W, 2 + o],
                        in1=dv, op=Alu.add,
                    )
                    ex = ep_pool.tile([128, _EP], f32, tag="ex", name="ex")
                    nc.scalar.activation(ex[P, 0:W], u2[P, 0:W], Act.Exp)
                    # S = EL0_o + e^x
                    nc.vector.tensor_tensor(
                        out=u2[P, 0:W], in0=Lbuf[P, ep0:ep0 + W, o],
                        in1=ex[P, 0:W], op=Alu.add,
                    )
                    nc.scalar.activation(ex[P, 0:W], u2[P, 0:W], Act.Ln)
                    # lpy_o = ln(S) - ln(1+e^d)
                    nc.vector.tensor_tensor(
                        out=outbuf[P].rearrange("p (t o) -> p t o", o=2)
                            [:, ep0:ep0 + W, o],
                        in0=ex[P, 0:W], in1=zq, op=Alu.subtract,
                    )
                ep0 += W

        nc.sync.dma_start(out.ap(), outbuf[:])

    nc.compile()
    return nc


def _prep_inputs(corr, kc, problem, A, trans_logits, obs_logits_problem,
                 init_logits, T):
    corr = np.asarray(corr).astype(np.int64)
    kc = np.asarray(kc).astype(np.int64)
    problem = np.asarray(problem).astype(np.int64)
    taba, tabp, gam0, a3p0 = _host_tables(
        np.asarray(A), np.asarray(trans_logits),
        np.asarray(obs_logits_problem), np.asarray(init_logits))

    in_maps = []
    for i in range(N_CORES):
        sl = slice(i * BL, (i + 1) * BL)
        kc_l = kc[sl, :T]
        pp_l = 2 * problem[sl, :T] + corr[sl, :T]
        kcw = _wrap_idx(kc_l.T.ravel())
        ppw = _wrap_idx(pp_l.T.ravel())
        in_maps.append({
            "taba": taba, "tabp": tabp, "kcw": kcw, "ppw": ppw,
            "gam0": gam0, "a3p0": a3p0,
        })
    return in_maps


def kernel(corr, kc, problem, A, trans_logits, obs_logits_problem, init_logits,
           _T=None, _trace=False):
    T = _T or T_FULL
    nc = _emit_program(T, min(_CHUNK, T))
    in_maps = _prep_inputs(corr, kc, problem, A, trans_logits,
                           obs_logits_problem, init_logits, T)

    from concourse.bass_utils import run_bass_kernel_spmd
    res = run_bass_kernel_spmd(nc, in_maps, core_ids=list(range(N_CORES)),
                               trace=_trace)
    outs = [r["out"].reshape(BL, T, 2) for r in res.results]
    full = np.concatenate(outs, axis=0).astype(np.float32)
    kernel.last_results = res
    return full


if __name__ == "__main__":
    pass
